# revision 12
# baseline (speedup 1.0000x reference)
"""Trainium2 Bass kernel for nn_CSFM_86011015070100 (topk_masking).

Data-parallel over batch: core b handles batch element b (B == 8 == n_cores).

Two launches per call:
  L1 (fused, single pass over x): streams pixel-bands of rgb+ir once,
     computing channel-sum maps (PE fp32 ones-matmul), channel-max maps
     (PE fp32 transpose + DVE/GpSimd reduce), per-channel sum(x^2) partials
     (ScalarE Square+accum), the 7x7 conv on-device (PE matmuls against
     host-built Toeplitz matrices), a ~1-ulp polynomial double sigmoid
     (exact range reduction + exp2 poly + DVE reciprocal), and the
     per-channel dot(sa, x_c) fine-grained partials -- all pipelined so x
     is read from HBM exactly once.
  host: f64 combine of partials -> sims -> stable argsort -> counts ->
     gather tables (numerically exact ordering; min sim gap ~7e-7 needs
     sa accurate to ~1 ulp, validated on device)
  L2: indirect-DMA channel gather of rgb/ir + add -> output
  host: fix up the single max-fused channel (when k_rgb != k_ir)
"""

import numpy as np
from contextlib import ExitStack

import concourse.bass as bass
import concourse.bacc as bacc
import concourse.tile as tile
from concourse import mybir
from concourse.bass_utils import run_bass_kernel_spmd
from concourse.masks import make_identity

F32 = mybir.dt.float32
I32 = mybir.dt.int32
A = mybir.AluOpType
ACT = mybir.ActivationFunctionType
AXX = mybir.AxisListType.X

B, C, H, W = 8, 256, 128, 128
HW = H * W          # 16384
NCORES = 8
CORE_IDS = list(range(NCORES))

P = 4096            # pixels per band (32 h-rows)
ROWS = P // W       # 32
NB = HW // P        # 4 bands
DBLK = 8            # dot partial grain
NDP = HW // DBLK    # 2048
SBLK = 1024         # square partial grain
NSP = HW // SBLK    # 16
GCHUNK = 2048       # pixels per gather chunk in L2
NGCH = HW // GCHUNK

LOG2E = 1.4426950408889634
MAGIC = 12582912.0  # 1.5 * 2^23
# minimax fit of (2^f - 1)/f on [-0.5, 0.5], degree 6 total (c1..c6)
SIGC = [0.693147181312687, 0.2402265084337212, 0.05550411058139838,
        0.009618129851338284, 0.001333378157866108, 0.00015403352087221094]

_cache = {}

TRACE = False
LAST_EXEC_NS = []
LAST_SADBG = []     # device-computed sa maps, for test diagnostics
LAST_VDBG = []
LAST_MDBG = []


def _run(nc, maps):
    try:
        r = run_bass_kernel_spmd(nc, maps, CORE_IDS, trace=TRACE)
    except Exception:
        import time

        time.sleep(2)
        r = run_bass_kernel_spmd(nc, maps, CORE_IDS, trace=TRACE)
    if r.exec_time_ns is not None:
        LAST_EXEC_NS.append(r.exec_time_ns)
    return r.results


# --------------------------------------------------------------------------
# device sigmoid: out = 1/(1 + 2^(-x*log2e)), ~1 ulp
# --------------------------------------------------------------------------
def _emit_sigmoid(nc, pool, out, in_ap, n, tb_bias=None):
    c1, c2, c3, c4, c5, c6 = SIGC
    t = pool.tile([128, n], F32, tag="sg_t", name="sg_t")
    rb = pool.tile([128, n], F32, tag="sg_rb", name="sg_rb")
    r = pool.tile([128, n], F32, tag="sg_r", name="sg_r")
    f = pool.tile([128, n], F32, tag="sg_f", name="sg_f")
    g = pool.tile([128, n], F32, tag="sg_g", name="sg_g")
    g2 = pool.tile([128, n], F32, tag="sg_g2", name="sg_g2")
    si = pool.tile([128, n], I32, tag="sg_si", name="sg_si")
    p = pool.tile([128, n], F32, tag="sg_p", name="sg_p")
    dd = pool.tile([128, n], F32, tag="sg_dd", name="sg_dd")
    if tb_bias is None:
        nc.scalar.activation(out=t[:], in_=in_ap, func=ACT.Copy, scale=-LOG2E)
    else:
        nc.scalar.activation(out=t[:], in_=in_ap, func=ACT.Identity,
                             scale=-LOG2E, bias=tb_bias)
    nc.scalar.activation(out=rb[:], in_=t[:], func=ACT.Copy, bias=MAGIC)
    nc.scalar.activation(out=r[:], in_=rb[:], func=ACT.Copy, bias=-MAGIC)
    nc.vector.tensor_tensor(out=f[:], in0=t[:], in1=r[:], op=A.subtract)
    nc.vector.tensor_scalar(out=g[:], in0=f[:], scalar1=c6, scalar2=None,
                            op0=A.mult)
    cur, nxt = g, g2
    for c in (c5, c4, c3, c2, c1):
        nc.vector.scalar_tensor_tensor(out=nxt[:], in0=cur[:], scalar=c,
                                       op0=A.add, in1=f[:], op1=A.mult)
        cur, nxt = nxt, cur
    nc.scalar.activation(out=rb[:], in_=r[:], func=ACT.Copy, scale=8388608.0,
                         bias=127.0 * 8388608.0)
    nc.scalar.copy(out=si[:], in_=rb[:])
    nc.vector.scalar_tensor_tensor(out=p[:], in0=cur[:], scalar=1.0,
                                   op0=A.add, in1=si[:].bitcast(F32),
                                   op1=A.mult)
    nc.scalar.activation(out=dd[:], in_=p[:], func=ACT.Copy, bias=1.0)
    nc.vector.reciprocal(out=out, in_=dd[:])


# --------------------------------------------------------------------------
# L1 fused: maps + on-device sa + dot/square partials, single pass over x
# --------------------------------------------------------------------------
def _build_l1():
    nc = bacc.Bacc("TRN2", target_bir_lowering=False, debug=False)
    rgb = nc.dram_tensor("rgb", [C, HW], F32, kind="ExternalInput").ap()
    ir = nc.dram_tensor("ir", [C, HW], F32, kind="ExternalInput").ap()
    convm = nc.dram_tensor("convm", [14, 128, 128], F32,
                           kind="ExternalInput").ap()
    cb = nc.dram_tensor("cb", [1, 1], F32, kind="ExternalInput").ap()
    dparts = nc.dram_tensor("dparts", [2, 2, 128, NDP], F32,
                            kind="ExternalOutput").ap()
    sparts = nc.dram_tensor("sparts", [2, 2, 128, NSP], F32,
                            kind="ExternalOutput").ap()
    sadbg = nc.dram_tensor("sadbg", [H, W], F32, kind="ExternalOutput").ap()
    vdbg = nc.dram_tensor("vdbg", [2, 2, 128, H + 6], F32,
                          kind="ExternalOutput").ap()
    mdbg = nc.dram_tensor("mdbg", [128, H], F32, kind="ExternalOutput").ap()

    xs = (rgb, ir)

    with tile.TileContext(nc) as tc, ExitStack() as ctx:
        consts = ctx.enter_context(tc.tile_pool(name="consts", bufs=1))
        xp = ctx.enter_context(tc.tile_pool(name="xp", bufs=2))
        prodp = ctx.enter_context(tc.tile_pool(name="prodp", bufs=2))
        sap = ctx.enter_context(tc.tile_pool(name="sap", bufs=2))
        vp = ctx.enter_context(tc.tile_pool(name="vp", bufs=1))
        stgp = ctx.enter_context(tc.tile_pool(name="stgp", bufs=4))
        sqp = ctx.enter_context(tc.tile_pool(name="sqp", bufs=1))
        sgp = ctx.enter_context(tc.tile_pool(name="sgp", bufs=2))
        dpp = ctx.enter_context(tc.tile_pool(name="dpp", bufs=2))
        spp = ctx.enter_context(tc.tile_pool(name="spp", bufs=1))
        ptp = ctx.enter_context(tc.tile_pool(name="ptp", bufs=1, space="PSUM"))
        sumsp = ctx.enter_context(
            tc.tile_pool(name="sumsp", bufs=2, space="PSUM"))
        convp = ctx.enter_context(
            tc.tile_pool(name="convp", bufs=1, space="PSUM"))

        ident = consts.tile([128, 128], F32)
        make_identity(nc, ident[:])
        ones = consts.tile([128, 1], F32)
        nc.vector.memset(ones[:], 1.0)
        cmt = consts.tile([128, 14 * 128], F32)
        convm_pkn = bass.AP(tensor=convm.tensor, offset=convm.offset,
                            ap=[[128, 128], [16384, 14], [1, 128]])
        nc.sync.dma_start(
            out=cmt[:].rearrange("p (k n) -> p k n", k=14), in_=convm_pkn)
        # conv bias broadcast [128,1], then tb = -log2e * b
        cbt = consts.tile([128, 1], F32)
        cb_b = bass.AP(tensor=cb.tensor, offset=cb.offset, ap=[[0, 128], [1, 1]])
        nc.sync.dma_start(out=cbt[:], in_=cb_b)
        tb = consts.tile([128, 1], F32)
        nc.scalar.activation(out=tb[:], in_=cbt[:], func=ACT.Copy, scale=-LOG2E)

        # per-modality maps, [w, h+6] with 3-col zero padding each side
        V = {}
        for m in range(2):
            for cix, nm in ((0, "avg"), (1, "max")):
                v = vp.tile([128, H + 6], F32, tag=f"V{m}{nm}",
                            name=f"V{m}{nm}")
                nc.vector.memset(v[:], 0.0)
                V[m, cix] = v

        sps = {}
        for m in range(2):
            for g in range(2):
                sps[m, g] = spp.tile([128, NSP], F32, tag=f"sp{m}{g}",
                                     name=f"sp{m}{g}")

        xt = {}  # live x tiles, keyed (mod, grp) -> tile of current band

        def emit_maps(b):
            for m in range(2):
                for g in range(2):
                    t = xp.tile([128, P], F32, tag=f"x{m}{g}", name=f"x{m}{g}")
                    nc.sync.dma_start(
                        out=t[:], in_=xs[m][g * 128:(g + 1) * 128,
                                            b * P:(b + 1) * P])
                    xt[m, g, b] = t
            xt.pop((0, 0, b - 2), None), xt.pop((0, 1, b - 2), None)
            xt.pop((1, 0, b - 2), None), xt.pop((1, 1, b - 2), None)
            for m in range(2):
                for q in range(8):  # quarters of the band: 4 px-blocks each
                    pt = ptp.tile([128, 4, 256], F32, tag=f"pt{m}",
                                  name=f"pt{m}")
                    for bb in range(4):
                        blk = q * 4 + bb
                        for g in range(2):
                            nc.tensor.transpose(
                                pt[:, bb, g * 128:(g + 1) * 128],
                                xt[m, g, b][:, blk * 128:(blk + 1) * 128],
                                ident[:])
                    col = 3 + b * ROWS + q * 4
                    nc.vector.tensor_reduce(out=V[m, 1][:, col:col + 4],
                                            in_=pt[:], axis=AXX, op=A.max)
                # channel sums -> psum [1,512] -> stage -> scatter to V
                for q8 in range(P // 512):
                    sl = slice(q8 * 512, (q8 + 1) * 512)
                    ps = sumsp.tile([1, 512], F32, tag="ps", name="ps")
                    nc.tensor.matmul(ps[:], ones[:], xt[m, 0, b][:, sl],
                                     start=True, stop=False)
                    nc.tensor.matmul(ps[:], ones[:], xt[m, 1, b][:, sl],
                                     start=False, stop=True)
                    stg = stgp.tile([1, 512], F32, tag="stg", name="stg")
                    nc.scalar.copy(out=stg[:], in_=ps[:])
                    for hh in range(4):
                        col = 3 + b * ROWS + q8 * 4 + hh
                        nc.scalar.dma_start(
                            out=V[m, 0][:, col:col + 1],
                            in_=stg[0:1, hh * 128:(hh + 1) * 128])
                # squares
                for g in range(2):
                    for i in range(P // SBLK):
                        sq = sqp.tile([128, SBLK], F32, tag="sq", name="sq")
                        pos = b * (P // SBLK) + i
                        nc.scalar.activation(
                            out=sq[:], in_=xt[m, g, b][:, i * SBLK:(i + 1) * SBLK],
                            func=ACT.Square,
                            accum_out=sps[m, g][:, pos:pos + 1])

        def emit_sa_dots(j):
            # conv: out^T[w,h] psum accum over (c,kh); both modalities in one
            # [128, 64] psum tile (cols 0:32 rgb, 32:64 ir)
            pcv = convp.tile([128, 64], F32, tag="pcv", name="pcv")
            for m in range(2):
                first, last = (0, 0), (1, 6)
                for cix in range(2):
                    for kh in range(7):
                        rhs = V[m, cix][:, j * ROWS + kh:j * ROWS + kh + ROWS]
                        nc.tensor.matmul(
                            pcv[:, m * ROWS:(m + 1) * ROWS],
                            cmt[:, (cix * 7 + kh) * 128:(cix * 7 + kh + 1) * 128],
                            rhs,
                            start=(cix, kh) == first, stop=(cix, kh) == last)
            cva = sgp.tile([128, ROWS], F32, tag="cva", name="cva")
            nc.scalar.copy(out=cva[:], in_=pcv[:, 0:ROWS])
            sM = sgp.tile([128, ROWS], F32, tag="sM", name="sM")
            nc.vector.tensor_tensor(out=sM[:], in0=cva[:],
                                    in1=pcv[:, ROWS:2 * ROWS], op=A.max)
            nc.scalar.dma_start(out=mdbg[:, j * ROWS:(j + 1) * ROWS],
                                in_=sM[:])
            y1 = sgp.tile([128, ROWS], F32, tag="y1", name="y1")
            _emit_sigmoid(nc, sgp, y1[:], sM[:], ROWS, tb_bias=tb[:, 0:1])
            saT = sgp.tile([128, ROWS], F32, tag="saT", name="saT")
            _emit_sigmoid(nc, sgp, saT[:], y1[:], ROWS)
            # transpose saT -> [ROWS, 128] (h-major pixel rows)
            psw = convp.tile([ROWS, 128], F32, tag="psw", name="psw")
            nc.tensor.transpose(psw[:], saT[:], ident[:])
            saw = sgp.tile([ROWS, 128], F32, tag="saw", name="saw")
            nc.scalar.copy(out=saw[:], in_=psw[:])
            nc.scalar.dma_start(out=sadbg[j * ROWS:(j + 1) * ROWS, :],
                                in_=saw[:])
            # broadcast each half-band to all 128 partitions (log doubling)
            for half in range(2):
                sa = sap.tile([128, P // 2], F32, tag="sa", name="sa")
                nc.scalar.dma_start(
                    out=sa[0:1, :],
                    in_=saw[half * 16:(half + 1) * 16, :])
                k = 1
                while k < 128:
                    nc.scalar.dma_start(out=sa[k:2 * k, :], in_=sa[0:k, :])
                    k *= 2
                # dots for this half-band
                for m in range(2):
                    for g in range(2):
                        prod = prodp.tile([128, P // 2], F32, tag="prod",
                                          name="prod")
                        nc.gpsimd.tensor_tensor(
                            out=prod[:],
                            in0=xt[m, g, j][:, half * (P // 2):(half + 1) * (P // 2)],
                            in1=sa[:], op=A.mult)
                        dp = dpp.tile([128, P // 2 // DBLK], F32, tag="dp",
                                      name="dp")
                        nc.vector.tensor_reduce(
                            out=dp[:],
                            in_=prod[:].rearrange("p (s q) -> p s q", q=DBLK),
                            axis=AXX, op=A.add)
                        off = j * (P // DBLK) + half * (P // 2 // DBLK)
                        nc.scalar.dma_start(
                            out=dparts[m, g][:, off:off + P // 2 // DBLK],
                            in_=dp[:])

        for b in range(NB):
            emit_maps(b)
            if b >= 1:
                emit_sa_dots(b - 1)
        emit_sa_dots(NB - 1)

        for m in range(2):
            for g in range(2):
                nc.scalar.dma_start(out=sparts[m, g], in_=sps[m, g][:])
                nc.scalar.dma_start(out=vdbg[m, g], in_=V[m, g][:])

    nc.compile()
    return nc


# --------------------------------------------------------------------------
# L2: gather channels of rgb/ir by index and add
# --------------------------------------------------------------------------
def _build_l2():
    nc = bacc.Bacc("TRN2", target_bir_lowering=False, debug=False,
                   num_swdge_queues=2)
    rgb = nc.dram_tensor("rgb", [C, HW], F32, kind="ExternalInput").ap()
    ir = nc.dram_tensor("ir", [C, HW], F32, kind="ExternalInput").ap()
    gidx = nc.dram_tensor("gidx", [2, C], I32, kind="ExternalInput").ap()
    out = nc.dram_tensor("out", [C, HW], F32, kind="ExternalOutput").ap()

    with tile.TileContext(nc) as tc, ExitStack() as ctx:
        idxp = ctx.enter_context(tc.tile_pool(name="idxp", bufs=1))
        rp = ctx.enter_context(tc.tile_pool(name="rp", bufs=6))
        ip = ctx.enter_context(tc.tile_pool(name="ip", bufs=6))
        op = ctx.enter_context(tc.tile_pool(name="op", bufs=6))

        for g in range(2):
            idr = idxp.tile([128, 1], I32, tag=f"idr{g}")
            idi = idxp.tile([128, 1], I32, tag=f"idi{g}")
            nc.sync.dma_start(out=idr[:], in_=gidx[0, g * 128:(g + 1) * 128])
            nc.sync.dma_start(out=idi[:], in_=gidx[1, g * 128:(g + 1) * 128])
            for ci in range(NGCH):
                sl = slice(ci * GCHUNK, (ci + 1) * GCHUNK)
                rt = rp.tile([128, GCHUNK], F32, tag="rt")
                it = ip.tile([128, GCHUNK], F32, tag="it")
                nc.gpsimd.indirect_dma_start(
                    out=rt[:], out_offset=None, in_=rgb,
                    in_offset=bass.IndirectOffsetOnAxis(ap=idr[:, 0:1], axis=0),
                    element_offset=ci * GCHUNK)
                inst = nc.gpsimd.indirect_dma_start(
                    out=it[:], out_offset=None, in_=ir,
                    in_offset=bass.IndirectOffsetOnAxis(ap=idi[:, 0:1], axis=0),
                    element_offset=ci * GCHUNK)
                inst.ins.queue = "qPoolDynamic1"  # second SWDGE ring
                ot = op.tile([128, GCHUNK], F32, tag="ot")
                nc.vector.tensor_tensor(out=ot[:], in0=rt[:], in1=it[:],
                                        op=A.add)
                nc.sync.dma_start(out=out[g * 128:(g + 1) * 128, sl], in_=ot[:])

    nc.compile()
    return nc


def _get(name, builder):
    if name not in _cache:
        _cache[name] = builder()
    return _cache[name]


def _make_convmat(conv_w):
    """Rt[c*7+kh][w', w] = wgt[c,kh, w'-w+3]; avg channel folded with /C."""
    cw = conv_w.astype(np.float64)[0].copy()  # [2,7,7]
    cw[0] /= C
    out = np.zeros((14, 128, 128), np.float32)
    wp = np.arange(128)[:, None]
    w = np.arange(128)[None, :]
    kw = wp - w + 3
    msk = (kw >= 0) & (kw <= 6)
    for c in range(2):
        for kh in range(7):
            row = cw[c, kh]
            out[c * 7 + kh][msk] = row[kw[msk]].astype(np.float32)
    return out


# --------------------------------------------------------------------------
# host glue
# --------------------------------------------------------------------------
def kernel(rgb, ir, conv_w, conv_b):
    rgb = np.ascontiguousarray(rgb, dtype=np.float32)
    ir = np.ascontiguousarray(ir, dtype=np.float32)
    conv_w = np.asarray(conv_w, dtype=np.float32)
    conv_b = np.asarray(conv_b, dtype=np.float32)

    rgb2 = rgb.reshape(B, C, HW)
    ir2 = ir.reshape(B, C, HW)
    LAST_EXEC_NS.clear()

    convm = _make_convmat(conv_w)
    cbv = conv_b.reshape(1, 1)

    # ---- L1 fused
    nc1 = _get("l1", _build_l1)
    maps1 = [{"rgb": rgb2[b], "ir": ir2[b], "convm": convm, "cb": cbv}
             for b in range(B)]
    res1 = _run(nc1, maps1)
    LAST_SADBG.clear()
    LAST_SADBG.extend(res1[b]["sadbg"] for b in range(B))
    LAST_VDBG.clear()
    LAST_VDBG.extend(res1[b]["vdbg"] for b in range(B))
    LAST_MDBG.clear()
    LAST_MDBG.extend(res1[b]["mdbg"] for b in range(B))

    # ---- host: sims, orders, counts, tables (f64 combine of partials)
    orders = np.zeros((B, 2, C), np.int64)
    cnts = np.zeros((B, 2), np.int64)
    for b in range(B):
        dparts = res1[b]["dparts"].astype(np.float64)  # [2,2,128,NDP]
        sparts = res1[b]["sparts"].astype(np.float64)  # [2,2,128,NSP]
        for t in range(2):
            dot = np.concatenate([dparts[t, 0].sum(-1), dparts[t, 1].sum(-1)])
            sq = np.concatenate([sparts[t, 0].sum(-1), sparts[t, 1].sum(-1)])
            tv = dot / np.maximum(np.sqrt(sq), 1e-30)
            orders[b, t] = np.argsort(tv, kind="stable")
            cnts[b, t] = int((tv > 0).sum())
    k_rgb = int(cnts[:, 0].max())
    k_ir = int(cnts[:, 1].max())
    ch = np.arange(C)
    src_rgb = ch.copy()
    src_ir = ch.copy()
    if k_rgb < k_ir:
        src_rgb[ch > k_rgb] -= 1
    elif k_ir < k_rgb:
        src_ir[ch > k_ir] -= 1

    # ---- L2
    nc2 = _get("l2", _build_l2)
    gidxs = []
    for b in range(B):
        g_r = orders[b, 0][src_rgb]
        g_i = orders[b, 1][src_ir]
        gidxs.append(np.stack([g_r, g_i]).astype(np.int32))
    maps3 = [{"rgb": rgb2[b], "ir": ir2[b], "gidx": gidxs[b]} for b in range(B)]
    res3 = _run(nc2, maps3)
    out = np.stack([res3[b]["out"].reshape(C, H, W) for b in range(B)])

    # ---- host fixup of the max-fused channel
    if k_rgb != k_ir:
        kpos = min(k_rgb, k_ir)
        for b in range(B):
            maxfea = np.maximum(rgb2[b, orders[b, 0][0]], ir2[b, orders[b, 1][0]])
            if k_rgb < k_ir:
                other = ir2[b, gidxs[b][1][kpos]]
            else:
                other = rgb2[b, gidxs[b][0][kpos]]
            out[b, kpos] = (maxfea + other).reshape(H, W)

    return out


# revision 15
# speedup vs baseline: 1.1755x; 1.1755x over previous
"""Trainium2 Bass kernel for nn_CSFM_86011015070100 (topk_masking).

Data-parallel over batch: core b handles batch element b (B == 8 == n_cores).

Two launches per call:
  L1 (fused, single pass over x): streams pixel-bands of rgb+ir once,
     computing channel-sum maps (PE fp32 ones-matmul), channel-max maps
     (PE fp32 transpose + DVE/GpSimd reduce), per-channel sum(x^2) partials
     (ScalarE Square+accum), the 7x7 conv on-device (PE matmuls against
     host-built Toeplitz matrices), a ~1-ulp polynomial double sigmoid
     (exact range reduction + exp2 poly + DVE reciprocal), and the
     per-channel dot(sa, x_c) fine-grained partials -- all pipelined so x
     is read from HBM exactly once.
  host: f64 combine of partials -> sims -> stable argsort -> counts ->
     gather tables (numerically exact ordering; min sim gap ~7e-7 needs
     sa accurate to ~1 ulp, validated on device)
  L2: indirect-DMA channel gather of rgb/ir + add -> output
  host: fix up the single max-fused channel (when k_rgb != k_ir)
"""

import numpy as np
from contextlib import ExitStack

import concourse.bass as bass
import concourse.bacc as bacc
import concourse.tile as tile
from concourse import mybir
from concourse.bass_utils import run_bass_kernel_spmd
from concourse.masks import make_identity

F32 = mybir.dt.float32
I32 = mybir.dt.int32
A = mybir.AluOpType
ACT = mybir.ActivationFunctionType
AXX = mybir.AxisListType.X

B, C, H, W = 8, 256, 128, 128
HW = H * W          # 16384
NCORES = 8
CORE_IDS = list(range(NCORES))

P = 4096            # pixels per band (32 h-rows)
ROWS = P // W       # 32
NB = HW // P        # 4 bands
DBLK = 8            # dot partial grain
NDP = HW // DBLK    # 2048
SBLK = 1024         # square partial grain
NSP = HW // SBLK    # 16
GCHUNK = 2048       # pixels per gather chunk in L2
NGCH = HW // GCHUNK

LOG2E = 1.4426950408889634
MAGIC = 12582912.0  # 1.5 * 2^23
# minimax fit of (2^f - 1)/f on [-0.5, 0.5], degree 6 total (c1..c6)
SIGC = [0.693147181312687, 0.2402265084337212, 0.05550411058139838,
        0.009618129851338284, 0.001333378157866108, 0.00015403352087221094]

_cache = {}

TRACE = False
LAST_EXEC_NS = []
LAST_SADBG = []     # device-computed sa maps, for test diagnostics
LAST_VDBG = []
LAST_MDBG = []


def _run(nc, maps):
    try:
        r = run_bass_kernel_spmd(nc, maps, CORE_IDS, trace=TRACE)
    except Exception:
        import time

        time.sleep(2)
        r = run_bass_kernel_spmd(nc, maps, CORE_IDS, trace=TRACE)
    if r.exec_time_ns is not None:
        LAST_EXEC_NS.append(r.exec_time_ns)
    return r.results


# --------------------------------------------------------------------------
# device sigmoid: out = 1/(1 + 2^(-x*log2e)), ~1 ulp
# --------------------------------------------------------------------------
def _emit_sigmoid(nc, pool, out, in_ap, n, bias_ap=None):
    """out = sigmoid(in + bias), ~1 ulp, all ops on DVE (no engine hops)."""
    c1, c2, c3, c4, c5, c6 = SIGC
    t = pool.tile([128, n], F32, tag="sg_t", name="sg_t")
    rb = pool.tile([128, n], F32, tag="sg_rb", name="sg_rb")
    r = pool.tile([128, n], F32, tag="sg_r", name="sg_r")
    f = pool.tile([128, n], F32, tag="sg_f", name="sg_f")
    g = pool.tile([128, n], F32, tag="sg_g", name="sg_g")
    g2 = pool.tile([128, n], F32, tag="sg_g2", name="sg_g2")
    si = pool.tile([128, n], I32, tag="sg_si", name="sg_si")
    p = pool.tile([128, n], F32, tag="sg_p", name="sg_p")
    dd = pool.tile([128, n], F32, tag="sg_dd", name="sg_dd")
    if bias_ap is None:
        nc.vector.tensor_scalar(out=t[:], in0=in_ap, scalar1=-LOG2E,
                                scalar2=None, op0=A.mult)
    else:
        nc.vector.tensor_scalar(out=t[:], in0=in_ap, scalar1=bias_ap,
                                scalar2=-LOG2E, op0=A.add, op1=A.mult)
    nc.vector.tensor_scalar(out=rb[:], in0=t[:], scalar1=MAGIC, scalar2=None,
                            op0=A.add)
    nc.vector.tensor_scalar(out=r[:], in0=rb[:], scalar1=-MAGIC, scalar2=None,
                            op0=A.add)
    nc.vector.tensor_tensor(out=f[:], in0=t[:], in1=r[:], op=A.subtract)
    nc.vector.tensor_scalar(out=g[:], in0=f[:], scalar1=c6, scalar2=None,
                            op0=A.mult)
    cur, nxt = g, g2
    for c in (c5, c4, c3, c2, c1):
        nc.vector.scalar_tensor_tensor(out=nxt[:], in0=cur[:], scalar=c,
                                       op0=A.add, in1=f[:], op1=A.mult)
        cur, nxt = nxt, cur
    nc.vector.tensor_scalar(out=rb[:], in0=r[:], scalar1=8388608.0,
                            scalar2=127.0 * 8388608.0, op0=A.mult, op1=A.add)
    nc.vector.tensor_scalar(out=si[:], in0=rb[:], scalar1=0.0, scalar2=None,
                            op0=A.add)
    nc.vector.scalar_tensor_tensor(out=p[:], in0=cur[:], scalar=1.0,
                                   op0=A.add, in1=si[:].bitcast(F32),
                                   op1=A.mult)
    nc.vector.tensor_scalar(out=dd[:], in0=p[:], scalar1=1.0, scalar2=None,
                            op0=A.add)
    nc.vector.reciprocal(out=out, in_=dd[:])


# --------------------------------------------------------------------------
# L1 fused: maps + on-device sa + dot/square partials, single pass over x
# --------------------------------------------------------------------------
Q = 1024            # pixels per quarter-band (8 h-rows)
NQ = P // Q         # 4


def _build_l1():
    nc = bacc.Bacc("TRN2", target_bir_lowering=False, debug=False)
    rgb = nc.dram_tensor("rgb", [C, HW], F32, kind="ExternalInput").ap()
    ir = nc.dram_tensor("ir", [C, HW], F32, kind="ExternalInput").ap()
    convm = nc.dram_tensor("convm", [14, 128, 128], F32,
                           kind="ExternalInput").ap()
    cb = nc.dram_tensor("cb", [1, 1], F32, kind="ExternalInput").ap()
    dparts = nc.dram_tensor("dparts", [2, 2, 128, NDP], F32,
                            kind="ExternalOutput").ap()
    sparts = nc.dram_tensor("sparts", [2, 2, 128, NSP], F32,
                            kind="ExternalOutput").ap()
    sadbg = nc.dram_tensor("sadbg", [H, W], F32, kind="ExternalOutput").ap()
    vdbg = nc.dram_tensor("vdbg", [2, 2, 128, H + 6], F32,
                          kind="ExternalOutput").ap()
    avrow = nc.dram_tensor("avrow", [2, P], F32, kind="Internal").ap()

    xs = (rgb, ir)

    with tile.TileContext(nc) as tc, ExitStack() as ctx:
        consts = ctx.enter_context(tc.tile_pool(name="consts", bufs=1))
        xp = ctx.enter_context(tc.tile_pool(name="xp", bufs=2))
        cmbp = ctx.enter_context(tc.tile_pool(name="cmbp", bufs=2))
        prodp = ctx.enter_context(tc.tile_pool(name="prodp", bufs=2))
        sap = ctx.enter_context(tc.tile_pool(name="sap", bufs=2))
        vp = ctx.enter_context(tc.tile_pool(name="vp", bufs=1))
        stgp = ctx.enter_context(tc.tile_pool(name="stgp", bufs=2))
        sqp = ctx.enter_context(tc.tile_pool(name="sqp", bufs=1))
        sgp = ctx.enter_context(tc.tile_pool(name="sgp", bufs=1))
        dpp = ctx.enter_context(tc.tile_pool(name="dpp", bufs=2))
        spp = ctx.enter_context(tc.tile_pool(name="spp", bufs=1))
        ptp = ctx.enter_context(tc.tile_pool(name="ptp", bufs=1, space="PSUM"))
        sumsp = ctx.enter_context(
            tc.tile_pool(name="sumsp", bufs=2, space="PSUM"))
        convp = ctx.enter_context(
            tc.tile_pool(name="convp", bufs=1, space="PSUM"))

        ident = consts.tile([128, 128], F32)
        make_identity(nc, ident[:])
        ones = consts.tile([128, 1], F32)
        nc.vector.memset(ones[:], 1.0)
        cmt = consts.tile([128, 14 * 128], F32)
        convm_pkn = bass.AP(tensor=convm.tensor, offset=convm.offset,
                            ap=[[128, 128], [16384, 14], [1, 128]])
        nc.sync.dma_start(
            out=cmt[:].rearrange("p (k n) -> p k n", k=14), in_=convm_pkn)
        cbt = consts.tile([128, 1], F32)
        cb_b = bass.AP(tensor=cb.tensor, offset=cb.offset, ap=[[0, 128], [1, 1]])
        nc.sync.dma_start(out=cbt[:], in_=cb_b)

        # per-modality maps, [w, h+6] with 3-col zero padding each side
        V = {}
        for m in range(2):
            for cix, nm in ((0, "avg"), (1, "max")):
                v = vp.tile([128, H + 6], F32, tag=f"V{m}{nm}",
                            name=f"V{m}{nm}")
                nc.vector.memset(v[:], 0.0)
                V[m, cix] = v

        sps = {}
        for m in range(2):
            for g in range(2):
                sps[m, g] = spp.tile([128, NSP], F32, tag=f"sp{m}{g}",
                                     name=f"sp{m}{g}")

        xt = {}

        def emit_maps(b):
            for m in range(2):
                for g in range(2):
                    t = xp.tile([128, P], F32, tag=f"x{m}{g}", name=f"x{m}{g}")
                    nc.sync.dma_start(
                        out=t[:], in_=xs[m][g * 128:(g + 1) * 128,
                                            b * P:(b + 1) * P])
                    xt[m, g, b] = t
                    xt.pop((m, g, b - 2), None)
            for m in range(2):
                # max maps: combine groups (DVE), transpose (PE), reduce (DVE)
                for q in range(NQ):
                    sl = slice(q * Q, (q + 1) * Q)
                    cmb = cmbp.tile([128, Q], F32, tag="cmb", name="cmb")
                    nc.vector.tensor_tensor(out=cmb[:], in0=xt[m, 0, b][:, sl],
                                            in1=xt[m, 1, b][:, sl], op=A.max)
                    pt = ptp.tile([128, 8, 128], F32, tag=f"pt{m}",
                                  name=f"pt{m}")
                    for bb in range(8):
                        nc.tensor.transpose(pt[:, bb],
                                            cmb[:, bb * 128:(bb + 1) * 128],
                                            ident[:])
                    col = 3 + b * ROWS + q * 8
                    nc.vector.tensor_reduce(out=V[m, 1][:, col:col + 8],
                                            in_=pt[:], axis=AXX, op=A.max)
                # channel sums: PE fp32 -> psum -> stage -> DRAM -> V_avg
                for q8 in range(P // 512):
                    sl = slice(q8 * 512, (q8 + 1) * 512)
                    ps = sumsp.tile([1, 512], F32, tag="ps", name="ps")
                    nc.tensor.matmul(ps[:], ones[:], xt[m, 0, b][:, sl],
                                     start=True, stop=False)
                    nc.tensor.matmul(ps[:], ones[:], xt[m, 1, b][:, sl],
                                     start=False, stop=True)
                    stg = stgp.tile([1, 512], F32, tag="stg", name="stg")
                    nc.scalar.copy(out=stg[:], in_=ps[:])
                    nc.scalar.dma_start(out=avrow[m, q8 * 512:(q8 + 1) * 512],
                                        in_=stg[:])
                av_src = bass.AP(tensor=avrow.tensor,
                                 offset=avrow.offset + m * P,
                                 ap=[[1, 128], [128, ROWS]])
                nc.scalar.dma_start(
                    out=V[m, 0][:, 3 + b * ROWS:3 + (b + 1) * ROWS],
                    in_=av_src)
                # squares
                for g in range(2):
                    for i in range(P // SBLK):
                        sq = sqp.tile([128, SBLK], F32, tag="sq", name="sq")
                        pos = b * (P // SBLK) + i
                        nc.scalar.activation(
                            out=sq[:],
                            in_=xt[m, g, b][:, i * SBLK:(i + 1) * SBLK],
                            func=ACT.Square,
                            accum_out=sps[m, g][:, pos:pos + 1])

        def emit_sa_dots(j):
            # conv: out^T[w,h] psum accum over (c,kh), both mods in one tile
            pcv = convp.tile([128, 64], F32, tag="pcv", name="pcv")
            for m in range(2):
                first, last = (0, 0), (1, 6)
                for cix in range(2):
                    for kh in range(7):
                        rhs = V[m, cix][:, j * ROWS + kh:j * ROWS + kh + ROWS]
                        nc.tensor.matmul(
                            pcv[:, m * ROWS:(m + 1) * ROWS],
                            cmt[:, (cix * 7 + kh) * 128:(cix * 7 + kh + 1) * 128],
                            rhs,
                            start=(cix, kh) == first, stop=(cix, kh) == last)
            cva = sgp.tile([128, ROWS], F32, tag="cva", name="cva")
            nc.scalar.copy(out=cva[:], in_=pcv[:, 0:ROWS])
            sM = sgp.tile([128, ROWS], F32, tag="sM", name="sM")
            nc.vector.tensor_tensor(out=sM[:], in0=cva[:],
                                    in1=pcv[:, ROWS:2 * ROWS], op=A.max)
            y1 = sgp.tile([128, ROWS], F32, tag="y1", name="y1")
            _emit_sigmoid(nc, sgp, y1[:], sM[:], ROWS, bias_ap=cbt[:, 0:1])
            saT = sgp.tile([128, ROWS], F32, tag="saT", name="saT")
            _emit_sigmoid(nc, sgp, saT[:], y1[:], ROWS)
            # transpose saT -> [ROWS, 128] (h-major pixel rows) -> DRAM
            psw = convp.tile([ROWS, 128], F32, tag="psw", name="psw")
            nc.tensor.transpose(psw[:], saT[:], ident[:])
            saw = sgp.tile([ROWS, 128], F32, tag="saw", name="saw")
            nc.scalar.copy(out=saw[:], in_=psw[:])
            nc.sync.dma_start(out=sadbg[j * ROWS:(j + 1) * ROWS, :],
                              in_=saw[:])
            # per quarter: broadcast sa from DRAM, products + dot partials
            for q in range(NQ):
                sa = sap.tile([128, Q], F32, tag="sa", name="sa")
                sa_src = bass.AP(
                    tensor=sadbg.tensor,
                    offset=sadbg.offset + j * P + q * Q,
                    ap=[[0, 128], [1, Q]])
                nc.sync.dma_start(out=sa[:], in_=sa_src)
                for m in range(2):
                    for g in range(2):
                        prod = prodp.tile([128, Q], F32, tag="prod",
                                          name="prod")
                        nc.gpsimd.tensor_tensor(
                            out=prod[:],
                            in0=xt[m, g, j][:, q * Q:(q + 1) * Q],
                            in1=sa[:], op=A.mult)
                        dpb = dpp.tile([128, Q // DBLK], F32, tag="dp",
                                       name="dp")
                        nc.vector.tensor_reduce(
                            out=dpb[:],
                            in_=prod[:].rearrange("p (s q) -> p s q", q=DBLK),
                            axis=AXX, op=A.add)
                        off = j * (P // DBLK) + q * (Q // DBLK)
                        nc.scalar.dma_start(
                            out=dparts[m, g][:, off:off + Q // DBLK],
                            in_=dpb[:])

        for b in range(NB):
            emit_maps(b)
            if b >= 1:
                emit_sa_dots(b - 1)
        emit_sa_dots(NB - 1)

        for m in range(2):
            for g in range(2):
                nc.scalar.dma_start(out=sparts[m, g], in_=sps[m, g][:])
                nc.scalar.dma_start(out=vdbg[m, g], in_=V[m, g][:])

    nc.compile()
    return nc


# --------------------------------------------------------------------------
# L2: gather channels of rgb/ir by index and add
# --------------------------------------------------------------------------
def _build_l2():
    nc = bacc.Bacc("TRN2", target_bir_lowering=False, debug=False,
                   num_swdge_queues=2)
    rgb = nc.dram_tensor("rgb", [C, HW], F32, kind="ExternalInput").ap()
    ir = nc.dram_tensor("ir", [C, HW], F32, kind="ExternalInput").ap()
    gidx = nc.dram_tensor("gidx", [2, C], I32, kind="ExternalInput").ap()
    out = nc.dram_tensor("out", [C, HW], F32, kind="ExternalOutput").ap()

    with tile.TileContext(nc) as tc, ExitStack() as ctx:
        idxp = ctx.enter_context(tc.tile_pool(name="idxp", bufs=1))
        rp = ctx.enter_context(tc.tile_pool(name="rp", bufs=6))
        ip = ctx.enter_context(tc.tile_pool(name="ip", bufs=6))
        op = ctx.enter_context(tc.tile_pool(name="op", bufs=6))

        for g in range(2):
            idr = idxp.tile([128, 1], I32, tag=f"idr{g}")
            idi = idxp.tile([128, 1], I32, tag=f"idi{g}")
            nc.sync.dma_start(out=idr[:], in_=gidx[0, g * 128:(g + 1) * 128])
            nc.sync.dma_start(out=idi[:], in_=gidx[1, g * 128:(g + 1) * 128])
            for ci in range(NGCH):
                sl = slice(ci * GCHUNK, (ci + 1) * GCHUNK)
                rt = rp.tile([128, GCHUNK], F32, tag="rt")
                it = ip.tile([128, GCHUNK], F32, tag="it")
                nc.gpsimd.indirect_dma_start(
                    out=rt[:], out_offset=None, in_=rgb,
                    in_offset=bass.IndirectOffsetOnAxis(ap=idr[:, 0:1], axis=0),
                    element_offset=ci * GCHUNK)
                inst = nc.gpsimd.indirect_dma_start(
                    out=it[:], out_offset=None, in_=ir,
                    in_offset=bass.IndirectOffsetOnAxis(ap=idi[:, 0:1], axis=0),
                    element_offset=ci * GCHUNK)
                inst.ins.queue = "qPoolDynamic1"  # second SWDGE ring
                ot = op.tile([128, GCHUNK], F32, tag="ot")
                nc.vector.tensor_tensor(out=ot[:], in0=rt[:], in1=it[:],
                                        op=A.add)
                nc.sync.dma_start(out=out[g * 128:(g + 1) * 128, sl], in_=ot[:])

    nc.compile()
    return nc


def _get(name, builder):
    if name not in _cache:
        _cache[name] = builder()
    return _cache[name]


def _make_convmat(conv_w):
    """Rt[c*7+kh][w', w] = wgt[c,kh, w'-w+3]; avg channel folded with /C."""
    cw = conv_w.astype(np.float64)[0].copy()  # [2,7,7]
    cw[0] /= C
    out = np.zeros((14, 128, 128), np.float32)
    wp = np.arange(128)[:, None]
    w = np.arange(128)[None, :]
    kw = wp - w + 3
    msk = (kw >= 0) & (kw <= 6)
    for c in range(2):
        for kh in range(7):
            row = cw[c, kh]
            out[c * 7 + kh][msk] = row[kw[msk]].astype(np.float32)
    return out


# --------------------------------------------------------------------------
# host glue
# --------------------------------------------------------------------------
def kernel(rgb, ir, conv_w, conv_b):
    rgb = np.ascontiguousarray(rgb, dtype=np.float32)
    ir = np.ascontiguousarray(ir, dtype=np.float32)
    conv_w = np.asarray(conv_w, dtype=np.float32)
    conv_b = np.asarray(conv_b, dtype=np.float32)

    rgb2 = rgb.reshape(B, C, HW)
    ir2 = ir.reshape(B, C, HW)
    LAST_EXEC_NS.clear()

    convm = _make_convmat(conv_w)
    cbv = conv_b.reshape(1, 1)

    # ---- L1 fused
    nc1 = _get("l1", _build_l1)
    maps1 = [{"rgb": rgb2[b], "ir": ir2[b], "convm": convm, "cb": cbv}
             for b in range(B)]
    res1 = _run(nc1, maps1)
    LAST_SADBG.clear()
    LAST_SADBG.extend(res1[b]["sadbg"] for b in range(B))
    LAST_VDBG.clear()
    LAST_VDBG.extend(res1[b]["vdbg"] for b in range(B))

    # ---- host: sims, orders, counts, tables (f64 combine of partials)
    orders = np.zeros((B, 2, C), np.int64)
    cnts = np.zeros((B, 2), np.int64)
    for b in range(B):
        dparts = res1[b]["dparts"].astype(np.float64)  # [2,2,128,NDP]
        sparts = res1[b]["sparts"].astype(np.float64)  # [2,2,128,NSP]
        for t in range(2):
            dot = np.concatenate([dparts[t, 0].sum(-1), dparts[t, 1].sum(-1)])
            sq = np.concatenate([sparts[t, 0].sum(-1), sparts[t, 1].sum(-1)])
            tv = dot / np.maximum(np.sqrt(sq), 1e-30)
            orders[b, t] = np.argsort(tv, kind="stable")
            cnts[b, t] = int((tv > 0).sum())
    k_rgb = int(cnts[:, 0].max())
    k_ir = int(cnts[:, 1].max())
    ch = np.arange(C)
    src_rgb = ch.copy()
    src_ir = ch.copy()
    if k_rgb < k_ir:
        src_rgb[ch > k_rgb] -= 1
    elif k_ir < k_rgb:
        src_ir[ch > k_ir] -= 1

    # ---- L2
    nc2 = _get("l2", _build_l2)
    gidxs = []
    for b in range(B):
        g_r = orders[b, 0][src_rgb]
        g_i = orders[b, 1][src_ir]
        gidxs.append(np.stack([g_r, g_i]).astype(np.int32))
    maps3 = [{"rgb": rgb2[b], "ir": ir2[b], "gidx": gidxs[b]} for b in range(B)]
    res3 = _run(nc2, maps3)
    out = np.stack([res3[b]["out"].reshape(C, H, W) for b in range(B)])

    # ---- host fixup of the max-fused channel
    if k_rgb != k_ir:
        kpos = min(k_rgb, k_ir)
        for b in range(B):
            maxfea = np.maximum(rgb2[b, orders[b, 0][0]], ir2[b, orders[b, 1][0]])
            if k_rgb < k_ir:
                other = ir2[b, gidxs[b][1][kpos]]
            else:
                other = rgb2[b, gidxs[b][0][kpos]]
            out[b, kpos] = (maxfea + other).reshape(H, W)

    return out


# revision 16
# speedup vs baseline: 1.2531x; 1.0660x over previous
"""Trainium2 Bass kernel for nn_CSFM_86011015070100 (topk_masking).

Data-parallel over batch: core b handles batch element b (B == 8 == n_cores).

Two launches per call:
  L1 (fused, single pass over x): streams pixel-bands of rgb+ir once,
     computing channel-sum maps (PE fp32 ones-matmul), channel-max maps
     (PE fp32 transpose + DVE/GpSimd reduce), per-channel sum(x^2) partials
     (ScalarE Square+accum), the 7x7 conv on-device (PE matmuls against
     host-built Toeplitz matrices), a ~1-ulp polynomial double sigmoid
     (exact range reduction + exp2 poly + DVE reciprocal), and the
     per-channel dot(sa, x_c) fine-grained partials -- all pipelined so x
     is read from HBM exactly once.
  host: f64 combine of partials -> sims -> stable argsort -> counts ->
     gather tables (numerically exact ordering; min sim gap ~7e-7 needs
     sa accurate to ~1 ulp, validated on device)
  L2: indirect-DMA channel gather of rgb/ir + add -> output
  host: fix up the single max-fused channel (when k_rgb != k_ir)
"""

import numpy as np
from contextlib import ExitStack

import concourse.bass as bass
import concourse.bacc as bacc
import concourse.tile as tile
from concourse import mybir
from concourse.bass_utils import run_bass_kernel_spmd
from concourse.masks import make_identity

F32 = mybir.dt.float32
I32 = mybir.dt.int32
A = mybir.AluOpType
ACT = mybir.ActivationFunctionType
AXX = mybir.AxisListType.X

B, C, H, W = 8, 256, 128, 128
HW = H * W          # 16384
NCORES = 8
CORE_IDS = list(range(NCORES))

P = 4096            # pixels per band (32 h-rows)
ROWS = P // W       # 32
NB = HW // P        # 4 bands
DBLK = 8            # dot partial grain
NDP = HW // DBLK    # 2048
SBLK = 1024         # square partial grain
NSP = HW // SBLK    # 16
GCHUNK = 2048       # pixels per gather chunk in L2
NGCH = HW // GCHUNK

LOG2E = 1.4426950408889634
MAGIC = 12582912.0  # 1.5 * 2^23
# minimax fit of (2^f - 1)/f on [-0.5, 0.5], degree 6 total (c1..c6)
SIGC = [0.693147181312687, 0.2402265084337212, 0.05550411058139838,
        0.009618129851338284, 0.001333378157866108, 0.00015403352087221094]

_cache = {}

TRACE = False
LAST_EXEC_NS = []
LAST_SADBG = []     # device-computed sa maps, for test diagnostics
LAST_VDBG = []
LAST_MDBG = []


def _run(nc, maps):
    try:
        r = run_bass_kernel_spmd(nc, maps, CORE_IDS, trace=TRACE)
    except Exception:
        import time

        time.sleep(2)
        r = run_bass_kernel_spmd(nc, maps, CORE_IDS, trace=TRACE)
    if r.exec_time_ns is not None:
        LAST_EXEC_NS.append(r.exec_time_ns)
    return r.results


# --------------------------------------------------------------------------
# device sigmoid: out = 1/(1 + 2^(-x*log2e)), ~1 ulp
# --------------------------------------------------------------------------
def _emit_sigmoid(nc, pool, out, in_ap, n, bias_ap=None):
    """out = sigmoid(in + bias), ~1 ulp, all ops on DVE (no engine hops)."""
    c1, c2, c3, c4, c5, c6 = SIGC
    t = pool.tile([128, n], F32, tag="sg_t", name="sg_t")
    rb = pool.tile([128, n], F32, tag="sg_rb", name="sg_rb")
    r = pool.tile([128, n], F32, tag="sg_r", name="sg_r")
    f = pool.tile([128, n], F32, tag="sg_f", name="sg_f")
    g = pool.tile([128, n], F32, tag="sg_g", name="sg_g")
    g2 = pool.tile([128, n], F32, tag="sg_g2", name="sg_g2")
    si = pool.tile([128, n], I32, tag="sg_si", name="sg_si")
    p = pool.tile([128, n], F32, tag="sg_p", name="sg_p")
    dd = pool.tile([128, n], F32, tag="sg_dd", name="sg_dd")
    if bias_ap is None:
        nc.vector.tensor_scalar(out=t[:], in0=in_ap, scalar1=-LOG2E,
                                scalar2=None, op0=A.mult)
    else:
        nc.vector.tensor_scalar(out=t[:], in0=in_ap, scalar1=bias_ap,
                                scalar2=-LOG2E, op0=A.add, op1=A.mult)
    nc.vector.tensor_scalar(out=rb[:], in0=t[:], scalar1=MAGIC, scalar2=None,
                            op0=A.add)
    nc.vector.tensor_scalar(out=r[:], in0=rb[:], scalar1=-MAGIC, scalar2=None,
                            op0=A.add)
    nc.vector.tensor_tensor(out=f[:], in0=t[:], in1=r[:], op=A.subtract)
    nc.vector.tensor_scalar(out=g[:], in0=f[:], scalar1=c6, scalar2=None,
                            op0=A.mult)
    cur, nxt = g, g2
    for c in (c5, c4, c3, c2, c1):
        nc.vector.scalar_tensor_tensor(out=nxt[:], in0=cur[:], scalar=c,
                                       op0=A.add, in1=f[:], op1=A.mult)
        cur, nxt = nxt, cur
    nc.vector.tensor_scalar(out=rb[:], in0=r[:], scalar1=8388608.0,
                            scalar2=127.0 * 8388608.0, op0=A.mult, op1=A.add)
    nc.vector.tensor_scalar(out=si[:], in0=rb[:], scalar1=0.0, scalar2=None,
                            op0=A.add)
    nc.vector.scalar_tensor_tensor(out=p[:], in0=cur[:], scalar=1.0,
                                   op0=A.add, in1=si[:].bitcast(F32),
                                   op1=A.mult)
    nc.vector.tensor_scalar(out=dd[:], in0=p[:], scalar1=1.0, scalar2=None,
                            op0=A.add)
    nc.vector.reciprocal(out=out, in_=dd[:])


# --------------------------------------------------------------------------
# L1 fused: maps + on-device sa + dot/square partials, single pass over x
# --------------------------------------------------------------------------
Q = 1024            # pixels per quarter-band (8 h-rows)
NQ = P // Q         # 4


def _build_l1():
    nc = bacc.Bacc("TRN2", target_bir_lowering=False, debug=False)
    rgb = nc.dram_tensor("rgb", [C, HW], F32, kind="ExternalInput").ap()
    ir = nc.dram_tensor("ir", [C, HW], F32, kind="ExternalInput").ap()
    convm = nc.dram_tensor("convm", [14, 128, 128], F32,
                           kind="ExternalInput").ap()
    cb = nc.dram_tensor("cb", [1, 1], F32, kind="ExternalInput").ap()
    dparts = nc.dram_tensor("dparts", [2, 2, 128, NDP], F32,
                            kind="ExternalOutput").ap()
    sparts = nc.dram_tensor("sparts", [2, 2, 128, NSP], F32,
                            kind="ExternalOutput").ap()
    sadbg = nc.dram_tensor("sadbg", [H, W], F32, kind="ExternalOutput").ap()
    vdbg = nc.dram_tensor("vdbg", [2, 2, 128, H + 6], F32,
                          kind="ExternalOutput").ap()
    avrow = nc.dram_tensor("avrow", [2, P], F32, kind="Internal").ap()

    xs = (rgb, ir)

    with tile.TileContext(nc) as tc, ExitStack() as ctx:
        consts = ctx.enter_context(tc.tile_pool(name="consts", bufs=1))
        xp = ctx.enter_context(tc.tile_pool(name="xp", bufs=2))
        cmbp = ctx.enter_context(tc.tile_pool(name="cmbp", bufs=2))
        prodp = ctx.enter_context(tc.tile_pool(name="prodp", bufs=2))
        sap = ctx.enter_context(tc.tile_pool(name="sap", bufs=2))
        vp = ctx.enter_context(tc.tile_pool(name="vp", bufs=1))
        stgp = ctx.enter_context(tc.tile_pool(name="stgp", bufs=2))
        sqp = ctx.enter_context(tc.tile_pool(name="sqp", bufs=1))
        sgp = ctx.enter_context(tc.tile_pool(name="sgp", bufs=1))
        dpp = ctx.enter_context(tc.tile_pool(name="dpp", bufs=2))
        spp = ctx.enter_context(tc.tile_pool(name="spp", bufs=1))
        ptp = ctx.enter_context(tc.tile_pool(name="ptp", bufs=1, space="PSUM"))
        sumsp = ctx.enter_context(
            tc.tile_pool(name="sumsp", bufs=2, space="PSUM"))
        convp = ctx.enter_context(
            tc.tile_pool(name="convp", bufs=1, space="PSUM"))

        ident = consts.tile([128, 128], F32)
        make_identity(nc, ident[:])
        ones = consts.tile([128, 1], F32)
        nc.vector.memset(ones[:], 1.0)
        cmt = consts.tile([128, 14 * 128], F32)
        convm_pkn = bass.AP(tensor=convm.tensor, offset=convm.offset,
                            ap=[[128, 128], [16384, 14], [1, 128]])
        nc.sync.dma_start(
            out=cmt[:].rearrange("p (k n) -> p k n", k=14), in_=convm_pkn)
        cbt = consts.tile([128, 1], F32)
        cb_b = bass.AP(tensor=cb.tensor, offset=cb.offset, ap=[[0, 128], [1, 1]])
        nc.sync.dma_start(out=cbt[:], in_=cb_b)

        # per-modality maps, [w, h+6] with 3-col zero padding each side
        V = {}
        for m in range(2):
            for cix, nm in ((0, "avg"), (1, "max")):
                v = vp.tile([128, H + 6], F32, tag=f"V{m}{nm}",
                            name=f"V{m}{nm}")
                nc.vector.memset(v[:], 0.0)
                V[m, cix] = v

        sps = {}
        for m in range(2):
            for g in range(2):
                sps[m, g] = spp.tile([128, NSP], F32, tag=f"sp{m}{g}",
                                     name=f"sp{m}{g}")

        xt = {}

        def emit_loads(b):
            for m in range(2):
                for g in range(2):
                    t = xp.tile([128, P], F32, tag=f"x{m}{g}", name=f"x{m}{g}")
                    nc.sync.dma_start(
                        out=t[:], in_=xs[m][g * 128:(g + 1) * 128,
                                            b * P:(b + 1) * P])
                    xt[m, g, b] = t
                    xt.pop((m, g, b - 2), None)

        def emit_maps_quarter(b, q):
            sl = slice(q * Q, (q + 1) * Q)
            for m in range(2):
                cmb = cmbp.tile([128, Q], F32, tag="cmb", name="cmb")
                nc.vector.tensor_tensor(out=cmb[:], in0=xt[m, 0, b][:, sl],
                                        in1=xt[m, 1, b][:, sl], op=A.max)
                pt = ptp.tile([128, 8, 128], F32, tag=f"pt{m}", name=f"pt{m}")
                for bb in range(8):
                    nc.tensor.transpose(pt[:, bb],
                                        cmb[:, bb * 128:(bb + 1) * 128],
                                        ident[:])
                col = 3 + b * ROWS + q * 8
                nc.vector.tensor_reduce(out=V[m, 1][:, col:col + 8],
                                        in_=pt[:], axis=AXX, op=A.max)
                for q8 in (2 * q, 2 * q + 1):
                    s8 = slice(q8 * 512, (q8 + 1) * 512)
                    ps = sumsp.tile([1, 512], F32, tag="ps", name="ps")
                    nc.tensor.matmul(ps[:], ones[:], xt[m, 0, b][:, s8],
                                     start=True, stop=False)
                    nc.tensor.matmul(ps[:], ones[:], xt[m, 1, b][:, s8],
                                     start=False, stop=True)
                    stg = stgp.tile([1, 512], F32, tag="stg", name="stg")
                    nc.scalar.copy(out=stg[:], in_=ps[:])
                    nc.scalar.dma_start(out=avrow[m, q8 * 512:(q8 + 1) * 512],
                                        in_=stg[:])
                av_src = bass.AP(tensor=avrow.tensor,
                                 offset=avrow.offset + m * P + q * Q,
                                 ap=[[1, 128], [128, 8]])
                nc.scalar.dma_start(out=V[m, 0][:, col:col + 8], in_=av_src)
                for g in range(2):
                    sq = sqp.tile([128, SBLK], F32, tag="sq", name="sq")
                    pos = b * (P // SBLK) + q
                    nc.scalar.activation(out=sq[:], in_=xt[m, g, b][:, sl],
                                         func=ACT.Square,
                                         accum_out=sps[m, g][:, pos:pos + 1])

        def emit_sa(j):
            pcv = convp.tile([128, 64], F32, tag="pcv", name="pcv")
            for m in range(2):
                first, last = (0, 0), (1, 6)
                for cix in range(2):
                    for kh in range(7):
                        rhs = V[m, cix][:, j * ROWS + kh:j * ROWS + kh + ROWS]
                        nc.tensor.matmul(
                            pcv[:, m * ROWS:(m + 1) * ROWS],
                            cmt[:, (cix * 7 + kh) * 128:(cix * 7 + kh + 1) * 128],
                            rhs,
                            start=(cix, kh) == first, stop=(cix, kh) == last)
            cva = sgp.tile([128, ROWS], F32, tag="cva", name="cva")
            nc.scalar.copy(out=cva[:], in_=pcv[:, 0:ROWS])
            sM = sgp.tile([128, ROWS], F32, tag="sM", name="sM")
            nc.vector.tensor_tensor(out=sM[:], in0=cva[:],
                                    in1=pcv[:, ROWS:2 * ROWS], op=A.max)
            y1 = sgp.tile([128, ROWS], F32, tag="y1", name="y1")
            _emit_sigmoid(nc, sgp, y1[:], sM[:], ROWS, bias_ap=cbt[:, 0:1])
            saT = sgp.tile([128, ROWS], F32, tag="saT", name="saT")
            _emit_sigmoid(nc, sgp, saT[:], y1[:], ROWS)
            psw = convp.tile([ROWS, 128], F32, tag="psw", name="psw")
            nc.tensor.transpose(psw[:], saT[:], ident[:])
            saw = sgp.tile([ROWS, 128], F32, tag="saw", name="saw")
            nc.scalar.copy(out=saw[:], in_=psw[:])
            nc.sync.dma_start(out=sadbg[j * ROWS:(j + 1) * ROWS, :],
                              in_=saw[:])

        def emit_dots_quarter(j, q):
            sa = sap.tile([128, Q], F32, tag="sa", name="sa")
            sa_src = bass.AP(tensor=sadbg.tensor,
                             offset=sadbg.offset + j * P + q * Q,
                             ap=[[0, 128], [1, Q]])
            nc.sync.dma_start(out=sa[:], in_=sa_src)
            for m in range(2):
                for g in range(2):
                    prod = prodp.tile([128, Q], F32, tag="prod", name="prod")
                    nc.gpsimd.tensor_tensor(
                        out=prod[:], in0=xt[m, g, j][:, q * Q:(q + 1) * Q],
                        in1=sa[:], op=A.mult)
                    dpb = dpp.tile([128, Q // DBLK], F32, tag="dp", name="dp")
                    nc.vector.tensor_reduce(
                        out=dpb[:],
                        in_=prod[:].rearrange("p (s q) -> p s q", q=DBLK),
                        axis=AXX, op=A.add)
                    off = j * (P // DBLK) + q * (Q // DBLK)
                    nc.sync.dma_start(
                        out=dparts[m, g][:, off:off + Q // DBLK], in_=dpb[:])

        for b in range(NB):
            emit_loads(b)
            emit_maps_quarter(b, 0)
            if b >= 1:
                emit_sa(b - 1)
            for q in range(1, NQ):
                if b >= 1:
                    emit_dots_quarter(b - 1, q - 1)
                emit_maps_quarter(b, q)
            if b >= 1:
                emit_dots_quarter(b - 1, NQ - 1)
        emit_sa(NB - 1)
        for q in range(NQ):
            emit_dots_quarter(NB - 1, q)

        for m in range(2):
            for g in range(2):
                nc.scalar.dma_start(out=sparts[m, g], in_=sps[m, g][:])
                nc.scalar.dma_start(out=vdbg[m, g], in_=V[m, g][:])

    nc.compile()
    return nc


# --------------------------------------------------------------------------
# L2: gather channels of rgb/ir by index and add
# --------------------------------------------------------------------------
def _build_l2():
    nc = bacc.Bacc("TRN2", target_bir_lowering=False, debug=False,
                   num_swdge_queues=2)
    rgb = nc.dram_tensor("rgb", [C, HW], F32, kind="ExternalInput").ap()
    ir = nc.dram_tensor("ir", [C, HW], F32, kind="ExternalInput").ap()
    gidx = nc.dram_tensor("gidx", [2, C], I32, kind="ExternalInput").ap()
    out = nc.dram_tensor("out", [C, HW], F32, kind="ExternalOutput").ap()

    with tile.TileContext(nc) as tc, ExitStack() as ctx:
        idxp = ctx.enter_context(tc.tile_pool(name="idxp", bufs=1))
        rp = ctx.enter_context(tc.tile_pool(name="rp", bufs=6))
        ip = ctx.enter_context(tc.tile_pool(name="ip", bufs=6))
        op = ctx.enter_context(tc.tile_pool(name="op", bufs=6))

        for g in range(2):
            idr = idxp.tile([128, 1], I32, tag=f"idr{g}")
            idi = idxp.tile([128, 1], I32, tag=f"idi{g}")
            nc.sync.dma_start(out=idr[:], in_=gidx[0, g * 128:(g + 1) * 128])
            nc.sync.dma_start(out=idi[:], in_=gidx[1, g * 128:(g + 1) * 128])
            for ci in range(NGCH):
                sl = slice(ci * GCHUNK, (ci + 1) * GCHUNK)
                rt = rp.tile([128, GCHUNK], F32, tag="rt")
                it = ip.tile([128, GCHUNK], F32, tag="it")
                nc.gpsimd.indirect_dma_start(
                    out=rt[:], out_offset=None, in_=rgb,
                    in_offset=bass.IndirectOffsetOnAxis(ap=idr[:, 0:1], axis=0),
                    element_offset=ci * GCHUNK)
                inst = nc.gpsimd.indirect_dma_start(
                    out=it[:], out_offset=None, in_=ir,
                    in_offset=bass.IndirectOffsetOnAxis(ap=idi[:, 0:1], axis=0),
                    element_offset=ci * GCHUNK)
                inst.ins.queue = "qPoolDynamic1"  # second SWDGE ring
                ot = op.tile([128, GCHUNK], F32, tag="ot")
                nc.vector.tensor_tensor(out=ot[:], in0=rt[:], in1=it[:],
                                        op=A.add)
                nc.sync.dma_start(out=out[g * 128:(g + 1) * 128, sl], in_=ot[:])

    nc.compile()
    return nc


def _get(name, builder):
    if name not in _cache:
        _cache[name] = builder()
    return _cache[name]


def _make_convmat(conv_w):
    """Rt[c*7+kh][w', w] = wgt[c,kh, w'-w+3]; avg channel folded with /C."""
    cw = conv_w.astype(np.float64)[0].copy()  # [2,7,7]
    cw[0] /= C
    out = np.zeros((14, 128, 128), np.float32)
    wp = np.arange(128)[:, None]
    w = np.arange(128)[None, :]
    kw = wp - w + 3
    msk = (kw >= 0) & (kw <= 6)
    for c in range(2):
        for kh in range(7):
            row = cw[c, kh]
            out[c * 7 + kh][msk] = row[kw[msk]].astype(np.float32)
    return out


# --------------------------------------------------------------------------
# host glue
# --------------------------------------------------------------------------
def kernel(rgb, ir, conv_w, conv_b):
    rgb = np.ascontiguousarray(rgb, dtype=np.float32)
    ir = np.ascontiguousarray(ir, dtype=np.float32)
    conv_w = np.asarray(conv_w, dtype=np.float32)
    conv_b = np.asarray(conv_b, dtype=np.float32)

    rgb2 = rgb.reshape(B, C, HW)
    ir2 = ir.reshape(B, C, HW)
    LAST_EXEC_NS.clear()

    convm = _make_convmat(conv_w)
    cbv = conv_b.reshape(1, 1)

    # ---- L1 fused
    nc1 = _get("l1", _build_l1)
    maps1 = [{"rgb": rgb2[b], "ir": ir2[b], "convm": convm, "cb": cbv}
             for b in range(B)]
    res1 = _run(nc1, maps1)
    LAST_SADBG.clear()
    LAST_SADBG.extend(res1[b]["sadbg"] for b in range(B))
    LAST_VDBG.clear()
    LAST_VDBG.extend(res1[b]["vdbg"] for b in range(B))

    # ---- host: sims, orders, counts, tables (f64 combine of partials)
    orders = np.zeros((B, 2, C), np.int64)
    cnts = np.zeros((B, 2), np.int64)
    for b in range(B):
        dparts = res1[b]["dparts"].astype(np.float64)  # [2,2,128,NDP]
        sparts = res1[b]["sparts"].astype(np.float64)  # [2,2,128,NSP]
        for t in range(2):
            dot = np.concatenate([dparts[t, 0].sum(-1), dparts[t, 1].sum(-1)])
            sq = np.concatenate([sparts[t, 0].sum(-1), sparts[t, 1].sum(-1)])
            tv = dot / np.maximum(np.sqrt(sq), 1e-30)
            orders[b, t] = np.argsort(tv, kind="stable")
            cnts[b, t] = int((tv > 0).sum())
    k_rgb = int(cnts[:, 0].max())
    k_ir = int(cnts[:, 1].max())
    ch = np.arange(C)
    src_rgb = ch.copy()
    src_ir = ch.copy()
    if k_rgb < k_ir:
        src_rgb[ch > k_rgb] -= 1
    elif k_ir < k_rgb:
        src_ir[ch > k_ir] -= 1

    # ---- L2
    nc2 = _get("l2", _build_l2)
    gidxs = []
    for b in range(B):
        g_r = orders[b, 0][src_rgb]
        g_i = orders[b, 1][src_ir]
        gidxs.append(np.stack([g_r, g_i]).astype(np.int32))
    maps3 = [{"rgb": rgb2[b], "ir": ir2[b], "gidx": gidxs[b]} for b in range(B)]
    res3 = _run(nc2, maps3)
    out = np.stack([res3[b]["out"].reshape(C, H, W) for b in range(B)])

    # ---- host fixup of the max-fused channel
    if k_rgb != k_ir:
        kpos = min(k_rgb, k_ir)
        for b in range(B):
            maxfea = np.maximum(rgb2[b, orders[b, 0][0]], ir2[b, orders[b, 1][0]])
            if k_rgb < k_ir:
                other = ir2[b, gidxs[b][1][kpos]]
            else:
                other = rgb2[b, gidxs[b][0][kpos]]
            out[b, kpos] = (maxfea + other).reshape(H, W)

    return out
